# revision 15
# baseline (speedup 1.0000x reference)
"""Multi-head attention on 8 Trainium2 NeuronCores (head-parallel).

Problem: Q,K,V [4096,512] fp32; Wq/Wk/Wv [8,512,64]; Wo [512,512].
  out = concat_h(softmax(QWq_h (KWk_h)^T / sqrt(64)) VWv_h) @ Wo

Sharding: one head per core. Each core computes its head end-to-end plus
its slice of the output projection (out_h @ Wo[64h:64h+64, :]); the host
sums the 8 partial [4096,512] outputs.

Per-core pipeline (n = 4096 queries, m = 4096 keys, d = 64):
  P1  projections (fp32 matmul). q and k are split hi/lo into bf16 pairs
      (q = q_hi + q_lo exactly in fp32; each part bf16) so the score
      matmuls can run at bf16 rate with ~fp32 accuracy via
      s = k_hi q_hi + k_lo q_hi + k_hi q_lo (the dropped lo*lo term is
      ~1e-3 absolute on logits of O(700)). v is evicted to bf16 tiles
      [128, d+1] with a constant ones column: the ones column makes the
      attn.V matmul also produce the softmax denominator.
      1/sqrt(d) is folded into Wq on the host.
  P2  per 512-query chunk, software-pipelined one chunk ahead:
      stats pass (bf16, 2-way row-packed): natural-layout scores
        [n-tile, m] -> per-row max (DVE reduce over PSUM); row maxes are
        DMA-scattered into row 64 of the q_hi operand.
      main pass (bf16 hi/lo): transposed scores; the hi*hi matmul carries
        a 65th contraction row (k side = -1, q side = rowmax) so the PSUM
        result is qk^T - rowmax directly; the two cross terms are K=64
        and run 2-way row-packed across m-tiles. ACT exp evicts
        PSUM -> SBUF bf16 attn^T.
      attn.V (bf16): accumulate outT [d+1, 512] in PSUM over all 32
        m-tiles; row d is the softmax sum.
  P3  Wo (bf16): partial[n-tile,512] = outT^T @ wo, scaled by 1/sum per
      query row (DVE per-partition scalar) on PSUM->SBUF eviction.

The double scores computation exists because softmax needs the query
index on partitions (per-partition reduce) while the attn.V matmul needs
the key index on partitions; scores are computed in both layouts (the
stats one only feeds the max, so it can be sloppy) rather than
transposing a 64MB attn matrix on-chip.

Row maxes ride in bf16: softmax is shift-invariant, so subtracting a
max that is off by <3 only scales exp values by <e^3, which the
self-consistent denominator (computed from the same bf16 attn weights)
cancels exactly.
"""

from contextlib import ExitStack

import numpy as np

N = 4096
DIM = 512
H = 8
D = 64
P = 128
CH = 512  # query columns per era (chunk)


def build_head_kernel(ctx, tc, outs, ins, n=N, dim=DIM, d=D):
    import concourse.bass as bass
    import concourse.mybir as mybir
    from concourse.bass import ts, ds

    nc = tc.nc
    f32 = mybir.dt.float32
    bf16 = mybir.dt.bfloat16
    AF = mybir.ActivationFunctionType

    KC = dim // P      # projection contraction chunks (4)
    NT = n // P        # 128-row tiles of n (= m tiles) (32)
    NCH = n // CH      # eras (8)
    NTC = CH // P      # n-tiles per era (4)
    MC = n // 512      # 512-wide m-chunks for the stats pass (8)
    PAIRS = MC // 2    # packed stats pairs per n-tile (4)
    GRP = NT // 2      # main groups per era, 2 m-tiles each (16)
    assert n % 1024 == 0 and dim % P == 0 and CH == 512

    qth_d, qtl_d = ins["QTH"], ins["QTL"]
    kth_d, ktl_d = ins["KTH"], ins["KTL"]
    vt_d = ins["VT"]
    wqh_d, wql_d = ins["wqh"], ins["wql"]
    wkh_d, wkl_d = ins["wkh"], ins["wkl"]
    wv_d, wo_d = ins["wv"], ins["wo"]
    out_d = outs["out"]

    singles = ctx.enter_context(tc.tile_pool(name="singles", bufs=1))

    # Persistent SBUF tensors.  *dup tiles hold the same data relocated to
    # partitions 64..127 so pairs of K<=64 matmuls can run concurrently in
    # distinct PE row-groups (tile_position row packing).
    Ah_ev = singles.tile([d + 1, n], bf16)  # q_hi; row d = rowmax (even eras)
    Ah_od = singles.tile([d + 1, n], bf16)  # q_hi; row d = rowmax (odd eras)
    Al = singles.tile([d, n], bf16)         # q_lo
    Bh = singles.tile([d + 1, n], bf16)     # k_hi; row d = -1
    Bl = singles.tile([d, n], bf16)         # k_lo
    qdup = singles.tile([P, n], bf16)       # rows 64..127 = q_hi
    kdup = singles.tile([P, n], bf16)       # rows 64..127 = k_hi
    qldup = singles.tile([P, n], bf16)      # rows 64..127 = q_lo
    kldup = singles.tile([P, n], bf16)      # rows 64..127 = k_lo
    v_sb = singles.tile([P, NT, d + 1], bf16)  # v tiles + ones column
    outT = singles.tile([d, n], bf16)       # attn_u @ v
    sumx = singles.tile([1, n], f32)        # softmax denominators
    rsum = singles.tile([P, NT], f32)       # sumexp gathered per n-tile
    rinv = singles.tile([P, NT], f32)
    wqh_sb = singles.tile([P, KC, d], bf16)
    wql_sb = singles.tile([P, KC, d], bf16)
    wkh_sb = singles.tile([P, KC, d], bf16)
    wkl_sb = singles.tile([P, KC, d], bf16)
    wv_sb = singles.tile([P, KC, d], f32)
    wo_sb = singles.tile([d, dim], f32)
    wo_bf = singles.tile([d, dim], bf16)

    # ---- constants / weight loads ----
    for w_sb, w_d in ((wqh_sb, wqh_d), (wql_sb, wql_d),
                      (wkh_sb, wkh_d), (wkl_sb, wkl_d), (wv_sb, wv_d)):
        nc.sync.dma_start(out=w_sb, in_=w_d.rearrange("(c p) e -> p c e", p=P))
    nc.sync.dma_start(out=wo_sb, in_=wo_d)
    nc.vector.tensor_copy(wo_bf, wo_sb)
    nc.vector.memset(Bh[d:d + 1, :], -1.0)
    nc.vector.memset(v_sb[:, :, d:d + 1], 1.0)

    # ---- P1: projections (fp32), evicted as bf16 hi/lo splits ----
    with tc.tile_pool(name="pstream", bufs=3) as pstream, \
         tc.tile_pool(name="pq_ps", bufs=2, space="PSUM") as pq_pool, \
         tc.tile_pool(name="pk_ps", bufs=2, space="PSUM") as pk_pool, \
         tc.tile_pool(name="pv_ps", bufs=2, space="PSUM") as pv_pool:
        for nb in range(n // 512):
            nbs = ds(nb * 512, 512)
            qth_t = pstream.tile([P, KC, 512], bf16, tag="qth_t")
            qtl_t = pstream.tile([P, KC, 512], bf16, tag="qtl_t")
            kth_t = pstream.tile([P, KC, 512], bf16, tag="kth_t")
            ktl_t = pstream.tile([P, KC, 512], bf16, tag="ktl_t")
            for t_sb, t_d in ((qth_t, qth_d), (qtl_t, qtl_d),
                              (kth_t, kth_d), (ktl_t, ktl_d)):
                nc.sync.dma_start(out=t_sb, in_=t_d[:, nbs].rearrange("(c p) x -> p c x", p=P))
            ps_q = pq_pool.tile([d, 512], f32)
            ps_k = pk_pool.tile([d, 512], f32)
            # q = Wq^T Q via bf16 hi/lo (lo*lo dropped)
            terms_q = [(wqh_sb, qth_t), (wqh_sb, qtl_t), (wql_sb, qth_t)]
            for i, (w, x) in enumerate(terms_q):
                for kc in range(KC):
                    nc.tensor.matmul(ps_q, lhsT=w[:, kc, :], rhs=x[:, kc, :],
                                     start=(i == 0 and kc == 0),
                                     stop=(i == 2 and kc == KC - 1))
            terms_k = [(wkh_sb, kth_t), (wkh_sb, ktl_t), (wkl_sb, kth_t)]
            for i, (w, x) in enumerate(terms_k):
                for kc in range(KC):
                    nc.tensor.matmul(ps_k, lhsT=w[:, kc, :], rhs=x[:, kc, :],
                                     start=(i == 0 and kc == 0),
                                     stop=(i == 2 and kc == KC - 1))
            nc.scalar.copy(Ah_ev[0:d, nbs], ps_q)                  # hi = bf16(q)
            nc.vector.tensor_sub(Al[:, nbs], ps_q, Ah_ev[0:d, nbs])  # lo = q - hi
            nc.scalar.copy(Bh[0:d, nbs], ps_k)
            nc.vector.tensor_sub(Bl[:, nbs], ps_k, Bh[0:d, nbs])
            for mt in range(4 * nb, 4 * nb + 4):
                mts = ts(mt, P)
                vt_t = pstream.tile([P, KC, P], f32, tag="vt_t")
                nc.sync.dma_start(out=vt_t, in_=vt_d[:, mts].rearrange("(c p) x -> p c x", p=P))
                ps_v = pv_pool.tile([P, d], f32)
                for kc in range(KC):
                    nc.tensor.matmul(ps_v, lhsT=vt_t[:, kc, :], rhs=wv_sb[:, kc, :],
                                     start=(kc == 0), stop=(kc == KC - 1))
                nc.vector.tensor_copy(v_sb[:, mt, 0:d], ps_v)
        # duplicate the odd-era q_hi and relocate hi/lo copies to partitions
        # 64..127 for row-packed matmuls (SBUF->SBUF DMA can cross partitions)
        nc.sync.dma_start(out=Ah_od[0:d, :], in_=Ah_ev[0:d, :])
        nc.sync.dma_start(out=qdup[d:2 * d, :], in_=Ah_ev[0:d, :])
        nc.sync.dma_start(out=kdup[d:2 * d, :], in_=Bh[0:d, :])
        nc.sync.dma_start(out=qldup[d:2 * d, :], in_=Al)
        nc.sync.dma_start(out=kldup[d:2 * d, :], in_=Bl)

    # ---- P2: stats (chunk c+1) interleaved with main (chunk c) ----
    with tc.tile_pool(name="st_ps_pool", bufs=1, space="PSUM") as st_pool, \
         tc.tile_pool(name="sc_ps_pool", bufs=2, space="PSUM") as sc_pool, \
         tc.tile_pool(name="av_ps_pool", bufs=2, space="PSUM") as av_pool, \
         tc.tile_pool(name="att_pool", bufs=3) as att_pool, \
         tc.tile_pool(name="nmax_pool", bufs=4) as nmax_pool:

        def stats_item(c, g):
            """One 2-way row-packed pair of natural-layout score matmuls."""
            j, p = divmod(g, PAIRS)
            gt = c * NTC + j  # global n-tile
            if p == 0:
                stats_item.nmax = nmax_pool.tile([P, PAIRS], bf16, tag="nmax")
            st_ps = st_pool.tile([P, 1024], f32)
            nc.tensor.matmul(st_ps[:, 0:512], lhsT=Ah_ev[0:d, ts(gt, P)],
                             rhs=Bh[0:d, ts(2 * p, 512)], start=True, stop=True)
            nc.tensor.matmul(st_ps[:, 512:1024], lhsT=qdup[d:2 * d, ts(gt, P)],
                             rhs=kdup[d:2 * d, ts(2 * p + 1, 512)], start=True, stop=True)
            nc.vector.reduce_max(stats_item.nmax[:, p:p + 1], st_ps,
                                 axis=mybir.AxisListType.X)
            if p == PAIRS - 1:
                if g == PAIRS - 1:  # first j of this chunk: allocate gather buf
                    stats_item.cmax = nmax_pool.tile([P, NTC], bf16, tag="cmax")
                nc.vector.reduce_max(stats_item.cmax[:, j:j + 1], stats_item.nmax,
                                     axis=mybir.AxisListType.X)
            if g == NTC * PAIRS - 1:
                At = Ah_ev if c % 2 == 0 else Ah_od
                # scatter per-row maxes into row d: column n = c*CH + jj*P + row
                for jj in range(NTC):
                    nc.sync.dma_start(out=At[d:d + 1, ds(c * CH + jj * P, P)],
                                      in_=stats_item.cmax[:, jj:jj + 1])

        def era(c):
            """Main pass for chunk c; stats for chunk c+1 interleaved."""
            At = Ah_ev if c % 2 == 0 else Ah_od
            cs = ds(c * CH, CH)
            r_hi65 = At[:, cs]        # [d+1, 512], row d = rowmax
            r_hi = At[0:d, cs]
            r_lo = Al[:, cs]
            r_hi_b = qdup[d:2 * d, cs]
            r_lo_b = qldup[d:2 * d, cs]
            av_ps = av_pool.tile([d + 1, 512], f32, tag="av")
            prev_att = None
            prev_g = -1

            def emit_av(att_t, g):
                nc.tensor.matmul(av_ps, lhsT=v_sb[:, 2 * g, :], rhs=att_t[:, 0:512],
                                 start=(g == 0), stop=False)
                nc.tensor.matmul(av_ps, lhsT=v_sb[:, 2 * g + 1, :], rhs=att_t[:, 512:1024],
                                 start=False, stop=(g == GRP - 1))

            for g in range(GRP):
                if c + 1 < NCH and g < NTC * PAIRS:
                    stats_item(c + 1, g)
                mta, mtb = ts(2 * g, P), ts(2 * g + 1, P)
                sc_ps = sc_pool.tile([P, 1024], f32, tag="sc")
                att_t = att_pool.tile([P, 1024], bf16, tag="att")
                sa, sb = sc_ps[:, 0:512], sc_ps[:, 512:1024]
                # hi*hi with the rowmax-subtraction row (K=65, unpackable)
                nc.tensor.matmul(sa, lhsT=Bh[:, mta], rhs=r_hi65, start=True, stop=False)
                nc.tensor.matmul(sb, lhsT=Bh[:, mtb], rhs=r_hi65, start=True, stop=False)
                # cross terms, 2-way row-packed (rows 0..63 / 64..127)
                nc.tensor.matmul(sa, lhsT=Bl[:, mta], rhs=r_hi, start=False, stop=False)
                nc.tensor.matmul(sb, lhsT=kldup[d:2 * d, mtb], rhs=r_hi_b,
                                 start=False, stop=False)
                nc.tensor.matmul(sa, lhsT=Bh[0:d, mta], rhs=r_lo, start=False, stop=True)
                nc.tensor.matmul(sb, lhsT=kdup[d:2 * d, mtb], rhs=r_lo_b,
                                 start=False, stop=True)
                nc.scalar.activation(att_t, sc_ps, AF.Exp)
                if prev_att is not None:
                    emit_av(prev_att, prev_g)
                prev_att, prev_g = att_t, g
            emit_av(prev_att, prev_g)
            # evict attn_u @ v (bf16) and the sumexp row (fp32), then gather
            # the per-n-tile denominators
            nc.scalar.copy(outT[:, cs], av_ps[0:d, :])
            nc.scalar.copy(sumx[:, cs], av_ps[d:d + 1, :])
            for jj in range(NTC):
                nc.sync.dma_start(out=rsum[:, c * NTC + jj:c * NTC + jj + 1],
                                  in_=sumx[:, ds(c * CH + jj * P, P)])
            nc.vector.reciprocal(rinv[:, ds(c * NTC, NTC)], rsum[:, ds(c * NTC, NTC)])

        for g in range(NTC * PAIRS):  # prologue: stats for chunk 0
            stats_item(0, g)
        for c in range(NCH):
            era(c)

    # ---- P3: output projection, scaled by 1/sumexp ----
    with tc.tile_pool(name="wo_ps_pool", bufs=4, space="PSUM") as wo_pool, \
         tc.tile_pool(name="out_pool", bufs=4) as out_pool:
        for t in range(NT):
            wo_ps = wo_pool.tile([P, dim], f32, tag="wo")
            nc.tensor.matmul(wo_ps, lhsT=outT[:, ts(t, P)], rhs=wo_bf,
                             start=True, stop=True)
            o_sb = out_pool.tile([P, dim], f32, tag="o")
            # alternate eviction engine so neither DVE nor ACT serializes P3
            if t % 2 == 0:
                nc.vector.tensor_scalar_mul(o_sb, wo_ps, rinv[:, t:t + 1])
            else:
                nc.scalar.mul(o_sb, wo_ps, rinv[:, t:t + 1])
            nc.sync.dma_start(out=out_d[ts(t, P), :], in_=o_sb)


def _hilo(x):
    """Split fp32 array into bf16 (hi, lo) with x ~= hi + lo."""
    import ml_dtypes

    hi = x.astype(ml_dtypes.bfloat16)
    lo = (x - hi.astype(np.float32)).astype(ml_dtypes.bfloat16)
    return np.ascontiguousarray(hi), np.ascontiguousarray(lo)


def make_in_maps(Q, K, V, Wq, Wk, Wv, Wo):
    """Host-side sharding: transpose activations, slice weights per head."""
    scale = 1.0 / np.sqrt(Wq.shape[-1])
    QTH, QTL = _hilo(np.ascontiguousarray(Q.T.astype(np.float32)))
    KTH, KTL = _hilo(np.ascontiguousarray(K.T.astype(np.float32)))
    VT = np.ascontiguousarray(V.T.astype(np.float32))
    d = Wq.shape[-1]
    in_maps = []
    for h in range(Wq.shape[0]):
        wqh, wql = _hilo(Wq[h].astype(np.float32) * scale)
        wkh, wkl = _hilo(Wk[h].astype(np.float32))
        in_maps.append({
            "QTH": QTH, "QTL": QTL, "KTH": KTH, "KTL": KTL, "VT": VT,
            "wqh": wqh, "wql": wql, "wkh": wkh, "wkl": wkl,
            "wv": np.ascontiguousarray(Wv[h].astype(np.float32)),
            "wo": np.ascontiguousarray(Wo[h * d:(h + 1) * d, :].astype(np.float32)),
        })
    return in_maps


_CACHE = {}


def _build_and_compile(n=N, dim=DIM, d=D, num_cores=H, repeats=1):
    import concourse.bass as bass
    import concourse.mybir as mybir
    import concourse.tile as tile
    from concourse import bacc

    key = (n, dim, d, num_cores, repeats)
    if key in _CACHE:
        return _CACHE[key]
    nc = bacc.Bacc("TRN2", target_bir_lowering=False, debug=False,
                   num_devices=num_cores)
    f32 = mybir.dt.float32
    bf16 = mybir.dt.bfloat16
    ins = {}
    for name in ("QTH", "QTL", "KTH", "KTL"):
        ins[name] = nc.dram_tensor(name, [dim, n], bf16, kind="ExternalInput").ap()
    ins["VT"] = nc.dram_tensor("VT", [dim, n], f32, kind="ExternalInput").ap()
    for name in ("wqh", "wql", "wkh", "wkl"):
        ins[name] = nc.dram_tensor(name, [dim, d], bf16, kind="ExternalInput").ap()
    ins["wv"] = nc.dram_tensor("wv", [dim, d], f32, kind="ExternalInput").ap()
    ins["wo"] = nc.dram_tensor("wo", [d, dim], f32, kind="ExternalInput").ap()
    outs = {"out": nc.dram_tensor("out", [n, dim], f32, kind="ExternalOutput").ap()}
    with tile.TileContext(nc) as tc:
        for _rep in range(repeats):
            with ExitStack() as ctx:
                build_head_kernel(ctx, tc, outs, ins, n=n, dim=dim, d=d)
    nc.compile()
    _CACHE[key] = nc
    return nc


def run_on_hw(in_maps, trace=False, **kwargs):
    from concourse.bass_utils import run_bass_kernel_spmd

    nc = _build_and_compile(num_cores=len(in_maps))
    return run_bass_kernel_spmd(nc, in_maps, core_ids=list(range(len(in_maps))),
                                trace=trace, **kwargs)


def kernel(Q, K, V, Wq, Wk, Wv, Wo):
    in_maps = make_in_maps(np.asarray(Q), np.asarray(K), np.asarray(V),
                           np.asarray(Wq), np.asarray(Wk), np.asarray(Wv),
                           np.asarray(Wo))
    res = run_on_hw(in_maps)
    out = np.zeros((N, DIM), dtype=np.float64)
    for r in res.results:
        out += r["out"].astype(np.float64)
    return out.astype(np.float32)


if __name__ == "__main__":
    rng = np.random.default_rng(0)
    inputs = {
        "Q": rng.standard_normal((N, DIM), dtype=np.float32),
        "K": rng.standard_normal((N, DIM), dtype=np.float32),
        "V": rng.standard_normal((N, DIM), dtype=np.float32),
        "Wq": rng.random((H, DIM, D), dtype=np.float32),
        "Wk": rng.random((H, DIM, D), dtype=np.float32),
        "Wv": rng.random((H, DIM, D), dtype=np.float32),
        "Wo": rng.random((DIM, DIM), dtype=np.float32),
    }
    out = kernel(**inputs)
    print(out.shape, out.dtype, np.abs(out).max())


# revision 22
# speedup vs baseline: 2.9292x; 2.9292x over previous
"""Multi-head attention on 8 Trainium2 NeuronCores (head-parallel).

Problem: Q,K,V [4096,512] fp32; Wq/Wk/Wv [8,512,64]; Wo [512,512].
  out = concat_h(softmax(QWq_h (KWk_h)^T / sqrt(64)) VWv_h) @ Wo

Sharding: one head per core. Each core computes its head end-to-end plus
its slice of the output projection (out_h @ Wo[64h:64h+64, :]); the host
sums the 8 partial [4096,512] outputs.

Per-core pipeline (n = 4096 queries, m = 4096 keys, d = 64):
  P1  projections (fp32 matmul). q and k are split hi/lo into bf16 pairs
      (q = q_hi + q_lo exactly in fp32; each part bf16) so the score
      matmuls can run at bf16 rate with ~fp32 accuracy via
      s = k_hi q_hi + k_lo q_hi + k_hi q_lo (the dropped lo*lo term is
      ~1e-3 absolute on logits of O(700)). v is evicted to bf16 tiles
      [128, d+1] with a constant ones column: the ones column makes the
      attn.V matmul also produce the softmax denominator.
      1/sqrt(d) is folded into Wq on the host.
  P2  per 512-query chunk, software-pipelined one chunk ahead:
      stats pass (bf16, 2-way row-packed): natural-layout scores
        [n-tile, m] -> per-row max (DVE reduce over PSUM); row maxes are
        DMA-scattered into row 64 of the q_hi operand.
      main pass (bf16 hi/lo): transposed scores; the hi*hi matmul carries
        a 65th contraction row (k side = -1, q side = rowmax) so the PSUM
        result is qk^T - rowmax directly; the two cross terms are K=64
        and run 2-way row-packed across m-tiles. ACT exp evicts
        PSUM -> SBUF bf16 attn^T.
      attn.V (bf16): accumulate outT [d+1, 512] in PSUM over all 32
        m-tiles; row d is the softmax sum.
  P3  Wo (bf16): partial[n-tile,512] = outT^T @ wo, scaled by 1/sum per
      query row (DVE per-partition scalar) on PSUM->SBUF eviction.

The double scores computation exists because softmax needs the query
index on partitions (per-partition reduce) while the attn.V matmul needs
the key index on partitions; scores are computed in both layouts (the
stats one only feeds the max, so it can be sloppy) rather than
transposing a 64MB attn matrix on-chip.

Row maxes ride in bf16: softmax is shift-invariant, so subtracting a
max that is off by <3 only scales exp values by <e^3, which the
self-consistent denominator (computed from the same bf16 attn weights)
cancels exactly.
"""

from contextlib import ExitStack

import numpy as np

N = 4096
DIM = 512
H = 8
D = 64
P = 128
CH = 512  # query columns per era (chunk)


def build_head_kernel(ctx, tc, outs, ins, n=N, dim=DIM, d=D):
    import concourse.bass as bass
    import concourse.mybir as mybir
    from concourse.bass import ts, ds

    nc = tc.nc
    f32 = mybir.dt.float32
    bf16 = mybir.dt.bfloat16
    AF = mybir.ActivationFunctionType

    KC = dim // P      # projection contraction chunks (4)
    NT = n // P        # 128-row tiles of n (= m tiles) (32)
    NCH = n // CH      # eras (8)
    NTC = CH // P      # n-tiles per era (4)
    MC = n // 512      # 512-wide m-chunks for the stats pass (8)
    PAIRS = MC // 2    # packed stats pairs per n-tile (4)
    GRP = NT // 2      # main groups per era, 2 m-tiles each (16)
    assert n % 1024 == 0 and dim % P == 0 and CH == 512

    qth_d, qtl_d = ins["QTH"], ins["QTL"]
    kth_d, ktl_d = ins["KTH"], ins["KTL"]
    vt_d = ins["VT"]
    wqh_d, wql_d = ins["wqh"], ins["wql"]
    wkh_d, wkl_d = ins["wkh"], ins["wkl"]
    wv_d, wo_d = ins["wv"], ins["wo"]
    out_d = outs["out"]

    singles = ctx.enter_context(tc.tile_pool(name="singles", bufs=1))

    # Persistent SBUF tensors.  *dup tiles hold the same data relocated to
    # partitions 64..127 so pairs of K<=64 matmuls can run concurrently in
    # distinct PE row-groups (tile_position row packing).
    Ah_ev = singles.tile([d + 1, n], bf16)  # q_hi; row d = rowmax (even eras)
    Ah_od = singles.tile([d + 1, n], bf16)  # q_hi; row d = rowmax (odd eras)
    Al = singles.tile([d, n], bf16)         # q_lo
    Bh = singles.tile([d + 1, n], bf16)     # k_hi; row d = -1
    Bl = singles.tile([d, n], bf16)         # k_lo
    qdup = singles.tile([P, n], bf16)       # rows 64..127 = q_hi
    kdup = singles.tile([P, n], bf16)       # rows 64..127 = k_hi
    qldup = singles.tile([P, n], bf16)      # rows 64..127 = q_lo
    kldup = singles.tile([P, n], bf16)      # rows 64..127 = k_lo
    v_sb = singles.tile([P, NT, d + 1], bf16)  # v tiles + ones column
    outT = singles.tile([d, n], bf16)       # attn_u @ v
    sumx = singles.tile([1, n], f32)        # softmax denominators
    rsum = singles.tile([P, NT], f32)       # sumexp gathered per n-tile
    rinv = singles.tile([P, NT], f32)
    wqh_sb = singles.tile([P, KC, d], bf16)
    wql_sb = singles.tile([P, KC, d], bf16)
    wkh_sb = singles.tile([P, KC, d], bf16)
    wkl_sb = singles.tile([P, KC, d], bf16)
    wv_sb = singles.tile([P, KC, d], f32)
    wo_sb = singles.tile([d, dim], f32)
    wo_bf = singles.tile([d, dim], bf16)

    # ---- constants / weight loads ----
    for w_sb, w_d in ((wqh_sb, wqh_d), (wql_sb, wql_d),
                      (wkh_sb, wkh_d), (wkl_sb, wkl_d), (wv_sb, wv_d)):
        nc.sync.dma_start(out=w_sb, in_=w_d.rearrange("(c p) e -> p c e", p=P))
    nc.sync.dma_start(out=wo_sb, in_=wo_d)
    nc.vector.tensor_copy(wo_bf, wo_sb)
    nc.vector.memset(Bh[d:d + 1, :], -1.0)
    nc.vector.memset(v_sb[:, :, d:d + 1], 1.0)

    # stats machinery: PSUM pool opens before P1 so chunk-0 stats can run
    # inside the (DMA-bound) projection phase as its k-chunks land
    st_pool = ctx.enter_context(tc.tile_pool(name="st_ps_pool", bufs=1, space="PSUM"))
    nmax_pool = ctx.enter_context(tc.tile_pool(name="nmax_pool", bufs=5))

    # stats nmax tiles are per-n-tile scratch; chunk-0 emission is p-major
    # (pair index advances as k-projection chunks complete), so all NTC nmax
    # tiles are live at once -- nmax_pool bufs covers NTC + the cmax tile
    nmax_tiles = {}

    def stats_item(c, g):
        """One 2-way row-packed pair of natural-layout score matmuls."""
        j, p = divmod(g, PAIRS)
        gt = c * NTC + j  # global n-tile
        if p == 0:
            nmax_tiles[j] = nmax_pool.tile([P, PAIRS], bf16, tag="nmax",
                                           name="nmax")
        st_ps = st_pool.tile([P, 1024], f32)
        nc.tensor.matmul(st_ps[:, 0:512], lhsT=Ah_ev[0:d, ts(gt, P)],
                         rhs=Bh[0:d, ts(2 * p, 512)], start=True, stop=True)
        nc.tensor.matmul(st_ps[:, 512:1024], lhsT=qdup[d:2 * d, ts(gt, P)],
                         rhs=kdup[d:2 * d, ts(2 * p + 1, 512)], start=True, stop=True)
        nc.vector.reduce_max(nmax_tiles[j][:, p:p + 1], st_ps,
                             axis=mybir.AxisListType.X)
        if p == PAIRS - 1:
            if j == 0:  # first finished n-tile of this chunk: alloc gather buf
                stats_item.cmax = nmax_pool.tile([P, NTC], bf16, tag="cmax")
            nc.vector.reduce_max(stats_item.cmax[:, j:j + 1], nmax_tiles[j],
                                 axis=mybir.AxisListType.X)
        if g == NTC * PAIRS - 1:
            At = Ah_ev if c % 2 == 0 else Ah_od
            # scatter per-row maxes into row d: column n = c*CH + jj*P + row
            for jj in range(NTC):
                nc.sync.dma_start(out=At[d:d + 1, ds(c * CH + jj * P, P)],
                                  in_=stats_item.cmax[:, jj:jj + 1])

    # ---- P1: projections (bf16 hi/lo), chunk-0 stats folded in ----
    NB = n // 512
    pending = []  # chunk-0 stats thunks, emitted at spaced slots for overlap

    def flush_one():
        if pending:
            pending.pop(0)()

    with tc.tile_pool(name="pstream", bufs=3) as pstream, \
         tc.tile_pool(name="pq_ps", bufs=2, space="PSUM") as pq_pool, \
         tc.tile_pool(name="pk_ps", bufs=2, space="PSUM") as pk_pool, \
         tc.tile_pool(name="pv_ps", bufs=2, space="PSUM") as pv_pool:

        def load_stream(t_d, tag, dtype, cols, nbs):
            t = pstream.tile([P, KC, cols], dtype, tag=tag, name=tag)
            nc.sync.dma_start(out=t, in_=t_d[:, nbs].rearrange("(c p) x -> p c x", p=P))
            return [t[:, kc, :] for kc in range(KC)]

        def v_tile(mt):
            vt_t = load_stream(vt_d, "vt", f32, P, ts(mt, P))
            ps_v = pv_pool.tile([P, d], f32)
            for kc in range(KC):
                nc.tensor.matmul(ps_v, lhsT=vt_t[kc], rhs=wv_sb[:, kc, :],
                                 start=(kc == 0), stop=(kc == KC - 1))
            nc.vector.tensor_copy(v_sb[:, mt, 0:d], ps_v)

        for nb in range(NB):
            nbs = ds(nb * 512, 512)
            qth_t = load_stream(qth_d, "qth", bf16, 512, nbs)
            qtl_t = load_stream(qtl_d, "qtl", bf16, 512, nbs)
            kth_t = load_stream(kth_d, "kth", bf16, 512, nbs)
            ktl_t = load_stream(ktl_d, "ktl", bf16, 512, nbs)
            flush_one()
            ps_q = pq_pool.tile([d, 512], f32)
            ps_k = pk_pool.tile([d, 512], f32)
            # q = Wq^T Q via bf16 hi/lo (lo*lo dropped)
            terms_q = [(wqh_sb, qth_t), (wqh_sb, qtl_t), (wql_sb, qth_t)]
            for i, (w, x) in enumerate(terms_q):
                for kc in range(KC):
                    nc.tensor.matmul(ps_q, lhsT=w[:, kc, :], rhs=x[kc],
                                     start=(i == 0 and kc == 0),
                                     stop=(i == 2 and kc == KC - 1))
            nc.scalar.copy(Ah_ev[0:d, nbs], ps_q)                  # hi = bf16(q)
            nc.vector.tensor_sub(Al[:, nbs], ps_q, Ah_ev[0:d, nbs])  # lo = q - hi
            flush_one()
            terms_k = [(wkh_sb, kth_t), (wkh_sb, ktl_t), (wkl_sb, kth_t)]
            for i, (w, x) in enumerate(terms_k):
                for kc in range(KC):
                    nc.tensor.matmul(ps_k, lhsT=w[:, kc, :], rhs=x[kc],
                                     start=(i == 0 and kc == 0),
                                     stop=(i == 2 and kc == KC - 1))
            nc.scalar.copy(Bh[0:d, nbs], ps_k)
            nc.vector.tensor_sub(Bl[:, nbs], ps_k, Bh[0:d, nbs])
            flush_one()
            # relocate this chunk's hi/lo copies to partitions 64..127
            # (SBUF->SBUF DMA can cross partitions; compute engines cannot)
            nc.sync.dma_start(out=qdup[d:2 * d, nbs], in_=Ah_ev[0:d, nbs])
            nc.sync.dma_start(out=kdup[d:2 * d, nbs], in_=Bh[0:d, nbs])
            nc.sync.dma_start(out=qldup[d:2 * d, nbs], in_=Al[:, nbs])
            nc.sync.dma_start(out=kldup[d:2 * d, nbs], in_=Bl[:, nbs])
            v_tile(2 * nb)
            flush_one()
            v_tile(2 * nb + 1)
            flush_one()
            if nb % 2 == 1:
                # k-chunks 2p, 2p+1 (p = nb//2) are now projected+relocated:
                # queue the chunk-0 stats pairs that contract against them
                p = nb // 2
                for j in range(NTC):
                    pending.append(lambda j=j, p=p: stats_item(0, j * PAIRS + p))
        for mt in range(2 * NB, NT):
            v_tile(mt)
            flush_one()
        while pending:
            flush_one()
        # odd-era copy of q_hi (separate tile so era c+1's rowmax scatter
        # never WARs era c's score matmul reads)
        nc.sync.dma_start(out=Ah_od[0:d, :], in_=Ah_ev[0:d, :])

    # ---- P2: stats (chunk c+1) interleaved with main (chunk c) ----
    with tc.tile_pool(name="sc_ps_pool", bufs=2, space="PSUM") as sc_pool, \
         tc.tile_pool(name="av_ps_pool", bufs=2, space="PSUM") as av_pool, \
         tc.tile_pool(name="att_pool", bufs=3) as att_pool:

        def era(c):
            """Main pass for chunk c; stats for chunk c+1 interleaved."""
            At = Ah_ev if c % 2 == 0 else Ah_od
            cs = ds(c * CH, CH)
            r_hi65 = At[:, cs]        # [d+1, 512], row d = rowmax
            r_hi = At[0:d, cs]
            r_lo = Al[:, cs]
            r_hi_b = qdup[d:2 * d, cs]
            r_lo_b = qldup[d:2 * d, cs]
            av_ps = av_pool.tile([d + 1, 512], f32, tag="av")
            prev_att = None
            prev_g = -1

            def emit_av(att_t, g):
                nc.tensor.matmul(av_ps, lhsT=v_sb[:, 2 * g, :], rhs=att_t[:, 0:512],
                                 start=(g == 0), stop=False)
                nc.tensor.matmul(av_ps, lhsT=v_sb[:, 2 * g + 1, :], rhs=att_t[:, 512:1024],
                                 start=False, stop=(g == GRP - 1))

            for g in range(GRP):
                if c + 1 < NCH and g < NTC * PAIRS:
                    stats_item(c + 1, g)
                mta, mtb = ts(2 * g, P), ts(2 * g + 1, P)
                sc_ps = sc_pool.tile([P, 1024], f32, tag="sc")
                att_t = att_pool.tile([P, 1024], bf16, tag="att")
                sa, sb = sc_ps[:, 0:512], sc_ps[:, 512:1024]
                # hi*hi with the rowmax-subtraction row (K=65, unpackable)
                nc.tensor.matmul(sa, lhsT=Bh[:, mta], rhs=r_hi65, start=True, stop=False)
                nc.tensor.matmul(sb, lhsT=Bh[:, mtb], rhs=r_hi65, start=True, stop=False)
                # cross terms, 2-way row-packed (rows 0..63 / 64..127)
                nc.tensor.matmul(sa, lhsT=Bl[:, mta], rhs=r_hi, start=False, stop=False)
                nc.tensor.matmul(sb, lhsT=kldup[d:2 * d, mtb], rhs=r_hi_b,
                                 start=False, stop=False)
                nc.tensor.matmul(sa, lhsT=Bh[0:d, mta], rhs=r_lo, start=False, stop=True)
                nc.tensor.matmul(sb, lhsT=kdup[d:2 * d, mtb], rhs=r_lo_b,
                                 start=False, stop=True)
                nc.scalar.activation(att_t, sc_ps, AF.Exp)
                if prev_att is not None:
                    emit_av(prev_att, prev_g)
                prev_att, prev_g = att_t, g
            emit_av(prev_att, prev_g)
            # evict attn_u @ v (bf16) and the sumexp row (fp32), then gather
            # the per-n-tile denominators
            nc.scalar.copy(outT[:, cs], av_ps[0:d, :])
            nc.scalar.copy(sumx[:, cs], av_ps[d:d + 1, :])
            for jj in range(NTC):
                nc.sync.dma_start(out=rsum[:, c * NTC + jj:c * NTC + jj + 1],
                                  in_=sumx[:, ds(c * CH + jj * P, P)])
            nc.vector.reciprocal(rinv[:, ds(c * NTC, NTC)], rsum[:, ds(c * NTC, NTC)])

        for c in range(NCH):
            era(c)

    # ---- P3: output projection, scaled by 1/sumexp ----
    with tc.tile_pool(name="wo_ps_pool", bufs=4, space="PSUM") as wo_pool, \
         tc.tile_pool(name="out_pool", bufs=4) as out_pool:
        for t in range(NT):
            wo_ps = wo_pool.tile([P, dim], f32, tag="wo")
            nc.tensor.matmul(wo_ps, lhsT=outT[:, ts(t, P)], rhs=wo_bf,
                             start=True, stop=True)
            o_sb = out_pool.tile([P, dim], f32, tag="o")
            # alternate eviction engine so neither DVE nor ACT serializes P3
            if t % 2 == 0:
                nc.vector.tensor_scalar_mul(o_sb, wo_ps, rinv[:, t:t + 1])
            else:
                nc.scalar.mul(o_sb, wo_ps, rinv[:, t:t + 1])
            nc.sync.dma_start(out=out_d[ts(t, P), :], in_=o_sb)


def _hilo(x):
    """Split fp32 array into bf16 (hi, lo) with x ~= hi + lo."""
    import ml_dtypes

    hi = x.astype(ml_dtypes.bfloat16)
    lo = (x - hi.astype(np.float32)).astype(ml_dtypes.bfloat16)
    return np.ascontiguousarray(hi), np.ascontiguousarray(lo)


def make_in_maps(Q, K, V, Wq, Wk, Wv, Wo):
    """Host-side sharding: transpose activations, slice weights per head."""
    scale = 1.0 / np.sqrt(Wq.shape[-1])
    QTH, QTL = _hilo(np.ascontiguousarray(Q.T.astype(np.float32)))
    KTH, KTL = _hilo(np.ascontiguousarray(K.T.astype(np.float32)))
    VT = np.ascontiguousarray(V.T.astype(np.float32))
    d = Wq.shape[-1]
    in_maps = []
    for h in range(Wq.shape[0]):
        wqh, wql = _hilo(Wq[h].astype(np.float32) * scale)
        wkh, wkl = _hilo(Wk[h].astype(np.float32))
        in_maps.append({
            "QTH": QTH, "QTL": QTL, "KTH": KTH, "KTL": KTL, "VT": VT,
            "wqh": wqh, "wql": wql, "wkh": wkh, "wkl": wkl,
            "wv": np.ascontiguousarray(Wv[h].astype(np.float32)),
            "wo": np.ascontiguousarray(Wo[h * d:(h + 1) * d, :].astype(np.float32)),
        })
    return in_maps


_CACHE = {}


def _build_and_compile(n=N, dim=DIM, d=D, num_cores=H, repeats=1):
    import concourse.bass as bass
    import concourse.mybir as mybir
    import concourse.tile as tile
    from concourse import bacc

    key = (n, dim, d, num_cores, repeats)
    if key in _CACHE:
        return _CACHE[key]
    nc = bacc.Bacc("TRN2", target_bir_lowering=False, debug=False,
                   num_devices=num_cores)
    f32 = mybir.dt.float32
    bf16 = mybir.dt.bfloat16
    ins = {}
    for name in ("QTH", "QTL", "KTH", "KTL"):
        ins[name] = nc.dram_tensor(name, [dim, n], bf16, kind="ExternalInput").ap()
    ins["VT"] = nc.dram_tensor("VT", [dim, n], f32, kind="ExternalInput").ap()
    for name in ("wqh", "wql", "wkh", "wkl"):
        ins[name] = nc.dram_tensor(name, [dim, d], bf16, kind="ExternalInput").ap()
    ins["wv"] = nc.dram_tensor("wv", [dim, d], f32, kind="ExternalInput").ap()
    ins["wo"] = nc.dram_tensor("wo", [d, dim], f32, kind="ExternalInput").ap()
    outs = {"out": nc.dram_tensor("out", [n, dim], f32, kind="ExternalOutput").ap()}
    with tile.TileContext(nc) as tc:
        for _rep in range(repeats):
            with ExitStack() as ctx:
                build_head_kernel(ctx, tc, outs, ins, n=n, dim=dim, d=d)
    nc.compile()
    _CACHE[key] = nc
    return nc


def run_on_hw(in_maps, trace=False, **kwargs):
    from concourse.bass_utils import run_bass_kernel_spmd

    nc = _build_and_compile(num_cores=len(in_maps))
    return run_bass_kernel_spmd(nc, in_maps, core_ids=list(range(len(in_maps))),
                                trace=trace, **kwargs)


def kernel(Q, K, V, Wq, Wk, Wv, Wo):
    in_maps = make_in_maps(np.asarray(Q), np.asarray(K), np.asarray(V),
                           np.asarray(Wq), np.asarray(Wk), np.asarray(Wv),
                           np.asarray(Wo))
    res = run_on_hw(in_maps)
    out = np.zeros((N, DIM), dtype=np.float64)
    for r in res.results:
        out += r["out"].astype(np.float64)
    return out.astype(np.float32)


if __name__ == "__main__":
    rng = np.random.default_rng(0)
    inputs = {
        "Q": rng.standard_normal((N, DIM), dtype=np.float32),
        "K": rng.standard_normal((N, DIM), dtype=np.float32),
        "V": rng.standard_normal((N, DIM), dtype=np.float32),
        "Wq": rng.random((H, DIM, D), dtype=np.float32),
        "Wk": rng.random((H, DIM, D), dtype=np.float32),
        "Wv": rng.random((H, DIM, D), dtype=np.float32),
        "Wo": rng.random((DIM, DIM), dtype=np.float32),
    }
    out = kernel(**inputs)
    print(out.shape, out.dtype, np.abs(out).max())


# revision 25
# speedup vs baseline: 2.9433x; 1.0048x over previous
"""Multi-head attention on 8 Trainium2 NeuronCores (head-parallel).

Problem: Q,K,V [4096,512] fp32; Wq/Wk/Wv [8,512,64]; Wo [512,512].
  out = concat_h(softmax(QWq_h (KWk_h)^T / sqrt(64)) VWv_h) @ Wo

Sharding: one head per core. Each core computes its head end-to-end plus
its slice of the output projection (out_h @ Wo[64h:64h+64, :]); the host
sums the 8 partial [4096,512] outputs.

Per-core pipeline (n = 4096 queries, m = 4096 keys, d = 64):
  P1  projections (fp32 matmul). q and k are split hi/lo into bf16 pairs
      (q = q_hi + q_lo exactly in fp32; each part bf16) so the score
      matmuls can run at bf16 rate with ~fp32 accuracy via
      s = k_hi q_hi + k_lo q_hi + k_hi q_lo (the dropped lo*lo term is
      ~1e-3 absolute on logits of O(700)). v is evicted to bf16 tiles
      [128, d+1] with a constant ones column: the ones column makes the
      attn.V matmul also produce the softmax denominator.
      1/sqrt(d) is folded into Wq on the host.
  P2  per 512-query chunk, software-pipelined one chunk ahead:
      stats pass (bf16, 2-way row-packed): natural-layout scores
        [n-tile, m] -> per-row max (DVE reduce over PSUM); row maxes are
        DMA-scattered into row 64 of the q_hi operand.
      main pass (bf16 hi/lo): transposed scores; the hi*hi matmul carries
        a 65th contraction row (k side = -1, q side = rowmax) so the PSUM
        result is qk^T - rowmax directly; the two cross terms are K=64
        and run 2-way row-packed across m-tiles. ACT exp evicts
        PSUM -> SBUF bf16 attn^T.
      attn.V (bf16): accumulate outT [d+1, 512] in PSUM over all 32
        m-tiles; row d is the softmax sum.
  P3  Wo (bf16): partial[n-tile,512] = outT^T @ wo, scaled by 1/sum per
      query row (DVE per-partition scalar) on PSUM->SBUF eviction.

The double scores computation exists because softmax needs the query
index on partitions (per-partition reduce) while the attn.V matmul needs
the key index on partitions; scores are computed in both layouts (the
stats one only feeds the max, so it can be sloppy) rather than
transposing a 64MB attn matrix on-chip.

Row maxes ride in bf16: softmax is shift-invariant, so subtracting a
max that is off by <3 only scales exp values by <e^3, which the
self-consistent denominator (computed from the same bf16 attn weights)
cancels exactly.
"""

from contextlib import ExitStack

import numpy as np

N = 4096
DIM = 512
H = 8
D = 64
P = 128
CH = 512  # query columns per era (chunk)


def build_head_kernel(ctx, tc, outs, ins, n=N, dim=DIM, d=D):
    import concourse.bass as bass
    import concourse.mybir as mybir
    from concourse.bass import ts, ds

    nc = tc.nc
    f32 = mybir.dt.float32
    bf16 = mybir.dt.bfloat16
    AF = mybir.ActivationFunctionType

    KC = dim // P      # projection contraction chunks (4)
    NT = n // P        # 128-row tiles of n (= m tiles) (32)
    NCH = n // CH      # eras (8)
    NTC = CH // P      # n-tiles per era (4)
    MC = n // 512      # 512-wide m-chunks for the stats pass (8)
    PAIRS = MC // 2    # packed stats pairs per n-tile (4)
    GRP = NT // 2      # main groups per era, 2 m-tiles each (16)
    assert n % 1024 == 0 and dim % P == 0 and CH == 512

    qth_d, qtl_d = ins["QTH"], ins["QTL"]
    kth_d, ktl_d = ins["KTH"], ins["KTL"]
    vt_d = ins["VT"]
    wqh_d, wql_d = ins["wqh"], ins["wql"]
    wkh_d, wkl_d = ins["wkh"], ins["wkl"]
    wv_d, wo_d = ins["wv"], ins["wo"]
    out_d = outs["out"]

    singles = ctx.enter_context(tc.tile_pool(name="singles", bufs=1))

    # Persistent SBUF tensors.  *dup tiles hold the same data relocated to
    # partitions 64..127 so pairs of K<=64 matmuls can run concurrently in
    # distinct PE row-groups (tile_position row packing).
    Ah_ev = singles.tile([d + 1, n], bf16)  # q_hi; row d = rowmax (even eras)
    Ah_od = singles.tile([d + 1, n], bf16)  # q_hi; row d = rowmax (odd eras)
    Al = singles.tile([d, n], bf16)         # q_lo
    Bh = singles.tile([d + 1, n], bf16)     # k_hi; row d = -1
    Bl = singles.tile([d, n], bf16)         # k_lo
    qdup = singles.tile([P, n], bf16)       # rows 64..127 = q_hi
    kdup = singles.tile([P, n], bf16)       # rows 64..127 = k_hi
    qldup = singles.tile([P, n], bf16)      # rows 64..127 = q_lo
    kldup = singles.tile([P, n], bf16)      # rows 64..127 = k_lo
    v_sb = singles.tile([P, NT, d + 1], bf16)  # v tiles + ones column
    outT = singles.tile([d, n], bf16)       # attn_u @ v
    sumx = singles.tile([1, n], f32)        # softmax denominators
    rsum = singles.tile([P, NT], f32)       # sumexp gathered per n-tile
    rinv = singles.tile([P, NT], f32)
    wqh_sb = singles.tile([P, KC, d], bf16)
    wql_sb = singles.tile([P, KC, d], bf16)
    wkh_sb = singles.tile([P, KC, d], bf16)
    wkl_sb = singles.tile([P, KC, d], bf16)
    wv_sb = singles.tile([P, KC, d], f32)
    wo_sb = singles.tile([d, dim], f32)
    wo_bf = singles.tile([d, dim], bf16)

    # ---- constants / weight loads ----
    for w_sb, w_d in ((wqh_sb, wqh_d), (wql_sb, wql_d),
                      (wkh_sb, wkh_d), (wkl_sb, wkl_d), (wv_sb, wv_d)):
        nc.sync.dma_start(out=w_sb, in_=w_d.rearrange("(c p) e -> p c e", p=P))
    nc.sync.dma_start(out=wo_sb, in_=wo_d)
    nc.vector.tensor_copy(wo_bf, wo_sb)
    nc.vector.memset(Bh[d:d + 1, :], -1.0)
    nc.vector.memset(v_sb[:, :, d:d + 1], 1.0)

    # stats machinery: PSUM pool opens before P1 so chunk-0 stats can run
    # inside the (DMA-bound) projection phase as its k-chunks land
    st_pool = ctx.enter_context(tc.tile_pool(name="st_ps_pool", bufs=1, space="PSUM"))
    nmax_pool = ctx.enter_context(tc.tile_pool(name="nmax_pool", bufs=5))

    # stats nmax tiles are per-n-tile scratch; chunk-0 emission is p-major
    # (pair index advances as k-projection chunks complete), so all NTC nmax
    # tiles are live at once -- nmax_pool bufs covers NTC + the cmax tile
    nmax_tiles = {}

    def stats_item(c, g):
        """One 2-way row-packed pair of natural-layout score matmuls."""
        j, p = divmod(g, PAIRS)
        gt = c * NTC + j  # global n-tile
        if p == 0:
            nmax_tiles[j] = nmax_pool.tile([P, PAIRS], bf16, tag="nmax",
                                           name="nmax")
        st_ps = st_pool.tile([P, 1024], f32)
        nc.tensor.matmul(st_ps[:, 0:512], lhsT=Ah_ev[0:d, ts(gt, P)],
                         rhs=Bh[0:d, ts(2 * p, 512)], start=True, stop=True)
        nc.tensor.matmul(st_ps[:, 512:1024], lhsT=qdup[d:2 * d, ts(gt, P)],
                         rhs=kdup[d:2 * d, ts(2 * p + 1, 512)], start=True, stop=True)
        nc.vector.reduce_max(nmax_tiles[j][:, p:p + 1], st_ps,
                             axis=mybir.AxisListType.X)
        if p == PAIRS - 1:
            if j == 0:  # first finished n-tile of this chunk: alloc gather buf
                stats_item.cmax = nmax_pool.tile([P, NTC], bf16, tag="cmax")
            nc.vector.reduce_max(stats_item.cmax[:, j:j + 1], nmax_tiles[j],
                                 axis=mybir.AxisListType.X)
        if g == NTC * PAIRS - 1:
            At = Ah_ev if c % 2 == 0 else Ah_od
            # scatter per-row maxes into row d: column n = c*CH + jj*P + row
            for jj in range(NTC):
                nc.sync.dma_start(out=At[d:d + 1, ds(c * CH + jj * P, P)],
                                  in_=stats_item.cmax[:, jj:jj + 1])

    # ---- P1: projections (bf16 hi/lo), chunk-0 stats folded in ----
    NB = n // 512
    pending = []  # chunk-0 stats thunks, emitted at spaced slots for overlap

    def flush_one():
        if pending:
            pending.pop(0)()

    with tc.tile_pool(name="pstream", bufs=3) as pstream, \
         tc.tile_pool(name="pq_ps", bufs=2, space="PSUM") as pq_pool, \
         tc.tile_pool(name="pk_ps", bufs=2, space="PSUM") as pk_pool, \
         tc.tile_pool(name="pv_ps", bufs=2, space="PSUM") as pv_pool:

        def load_stream(t_d, tag, dtype, cols, nbs):
            t = pstream.tile([P, KC, cols], dtype, tag=tag, name=tag)
            nc.sync.dma_start(out=t, in_=t_d[:, nbs].rearrange("(c p) x -> p c x", p=P))
            return [t[:, kc, :] for kc in range(KC)]

        def v_tile(mt):
            vt_t = load_stream(vt_d, "vt", f32, P, ts(mt, P))
            ps_v = pv_pool.tile([P, d], f32)
            for kc in range(KC):
                nc.tensor.matmul(ps_v, lhsT=vt_t[kc], rhs=wv_sb[:, kc, :],
                                 start=(kc == 0), stop=(kc == KC - 1))
            nc.vector.tensor_copy(v_sb[:, mt, 0:d], ps_v)

        for nb in range(NB):
            nbs = ds(nb * 512, 512)
            qth_t = load_stream(qth_d, "qth", bf16, 512, nbs)
            qtl_t = load_stream(qtl_d, "qtl", bf16, 512, nbs)
            kth_t = load_stream(kth_d, "kth", bf16, 512, nbs)
            ktl_t = load_stream(ktl_d, "ktl", bf16, 512, nbs)
            flush_one()
            ps_q = pq_pool.tile([d, 512], f32)
            ps_k = pk_pool.tile([d, 512], f32)
            # q = Wq^T Q via bf16 hi/lo (lo*lo dropped)
            terms_q = [(wqh_sb, qth_t), (wqh_sb, qtl_t), (wql_sb, qth_t)]
            for i, (w, x) in enumerate(terms_q):
                for kc in range(KC):
                    nc.tensor.matmul(ps_q, lhsT=w[:, kc, :], rhs=x[kc],
                                     start=(i == 0 and kc == 0),
                                     stop=(i == 2 and kc == KC - 1))
            nc.scalar.copy(Ah_ev[0:d, nbs], ps_q)                  # hi = bf16(q)
            nc.vector.tensor_sub(Al[:, nbs], ps_q, Ah_ev[0:d, nbs])  # lo = q - hi
            flush_one()
            terms_k = [(wkh_sb, kth_t), (wkh_sb, ktl_t), (wkl_sb, kth_t)]
            for i, (w, x) in enumerate(terms_k):
                for kc in range(KC):
                    nc.tensor.matmul(ps_k, lhsT=w[:, kc, :], rhs=x[kc],
                                     start=(i == 0 and kc == 0),
                                     stop=(i == 2 and kc == KC - 1))
            nc.scalar.copy(Bh[0:d, nbs], ps_k)
            nc.vector.tensor_sub(Bl[:, nbs], ps_k, Bh[0:d, nbs])
            flush_one()
            # relocate this chunk's hi/lo copies to partitions 64..127
            # (SBUF->SBUF DMA can cross partitions; compute engines cannot)
            nc.sync.dma_start(out=qdup[d:2 * d, nbs], in_=Ah_ev[0:d, nbs])
            nc.sync.dma_start(out=kdup[d:2 * d, nbs], in_=Bh[0:d, nbs])
            nc.sync.dma_start(out=qldup[d:2 * d, nbs], in_=Al[:, nbs])
            nc.sync.dma_start(out=kldup[d:2 * d, nbs], in_=Bl[:, nbs])
            v_tile(2 * nb)
            flush_one()
            v_tile(2 * nb + 1)
            flush_one()
            if nb % 2 == 1:
                # k-chunks 2p, 2p+1 (p = nb//2) are now projected+relocated:
                # queue the chunk-0 stats pairs that contract against them
                p = nb // 2
                for j in range(NTC):
                    pending.append(lambda j=j, p=p: stats_item(0, j * PAIRS + p))
        for mt in range(2 * NB, NT):
            v_tile(mt)
            flush_one()
        while pending:
            flush_one()
        # odd-era copy of q_hi (separate tile so era c+1's rowmax scatter
        # never WARs era c's score matmul reads)
        nc.sync.dma_start(out=Ah_od[0:d, :], in_=Ah_ev[0:d, :])

    # ---- P2: stats (chunk c+1) interleaved with main (chunk c) ----
    with tc.tile_pool(name="sc_ps_pool", bufs=2, space="PSUM") as sc_pool, \
         tc.tile_pool(name="av_ps_pool", bufs=2, space="PSUM") as av_pool, \
         tc.tile_pool(name="att_pool", bufs=5) as att_pool:

        def era(c):
            """Main pass for chunk c; stats for chunk c+1 interleaved."""
            At = Ah_ev if c % 2 == 0 else Ah_od
            cs = ds(c * CH, CH)
            r_hi65 = At[:, cs]        # [d+1, 512], row d = rowmax
            r_hi = At[0:d, cs]
            r_lo = Al[:, cs]
            r_hi_b = qdup[d:2 * d, cs]
            r_lo_b = qldup[d:2 * d, cs]
            av_ps = av_pool.tile([d + 1, 512], f32, tag="av")
            att_fifo = []  # (att_tile, g) awaiting attn.V, deferred 2 groups

            def emit_av(att_t, g):
                nc.tensor.matmul(av_ps, lhsT=v_sb[:, 2 * g, :], rhs=att_t[:, 0:512],
                                 start=(g == 0), stop=False)
                nc.tensor.matmul(av_ps, lhsT=v_sb[:, 2 * g + 1, :], rhs=att_t[:, 512:1024],
                                 start=False, stop=(g == GRP - 1))

            for g in range(GRP):
                if c + 1 < NCH and g < NTC * PAIRS:
                    stats_item(c + 1, g)
                mta, mtb = ts(2 * g, P), ts(2 * g + 1, P)
                sc_ps = sc_pool.tile([P, 1024], f32, tag="sc")
                att_t = att_pool.tile([P, 1024], bf16, tag="att")
                sa, sb = sc_ps[:, 0:512], sc_ps[:, 512:1024]
                # hi*hi with the rowmax-subtraction row (K=65, unpackable)
                nc.tensor.matmul(sa, lhsT=Bh[:, mta], rhs=r_hi65, start=True, stop=False)
                nc.tensor.matmul(sb, lhsT=Bh[:, mtb], rhs=r_hi65, start=True, stop=False)
                # cross terms, 2-way row-packed (rows 0..63 / 64..127)
                nc.tensor.matmul(sa, lhsT=Bl[:, mta], rhs=r_hi, start=False, stop=False)
                nc.tensor.matmul(sb, lhsT=kldup[d:2 * d, mtb], rhs=r_hi_b,
                                 start=False, stop=False)
                nc.tensor.matmul(sa, lhsT=Bh[0:d, mta], rhs=r_lo, start=False, stop=True)
                nc.tensor.matmul(sb, lhsT=kdup[d:2 * d, mtb], rhs=r_lo_b,
                                 start=False, stop=True)
                nc.scalar.activation(att_t, sc_ps, AF.Exp)
                # defer attn.V two groups so the exp it reads is long done
                # even when row-packing makes PE outpace ACT on real hardware
                att_fifo.append((att_t, g))
                if len(att_fifo) > 2:
                    emit_av(*att_fifo.pop(0))
            for item in att_fifo:
                emit_av(*item)
            # evict attn_u @ v (bf16) and the sumexp row (fp32), then gather
            # the per-n-tile denominators
            nc.scalar.copy(outT[:, cs], av_ps[0:d, :])
            nc.scalar.copy(sumx[:, cs], av_ps[d:d + 1, :])
            for jj in range(NTC):
                nc.sync.dma_start(out=rsum[:, c * NTC + jj:c * NTC + jj + 1],
                                  in_=sumx[:, ds(c * CH + jj * P, P)])
            nc.vector.reciprocal(rinv[:, ds(c * NTC, NTC)], rsum[:, ds(c * NTC, NTC)])

        for c in range(NCH):
            era(c)

    # ---- P3: output projection, scaled by 1/sumexp ----
    with tc.tile_pool(name="wo_ps_pool", bufs=4, space="PSUM") as wo_pool, \
         tc.tile_pool(name="out_pool", bufs=4) as out_pool:
        for t in range(NT):
            wo_ps = wo_pool.tile([P, dim], f32, tag="wo")
            nc.tensor.matmul(wo_ps, lhsT=outT[:, ts(t, P)], rhs=wo_bf,
                             start=True, stop=True)
            o_sb = out_pool.tile([P, dim], f32, tag="o")
            # alternate eviction engine so neither DVE nor ACT serializes P3
            if t % 2 == 0:
                nc.vector.tensor_scalar_mul(o_sb, wo_ps, rinv[:, t:t + 1])
            else:
                nc.scalar.mul(o_sb, wo_ps, rinv[:, t:t + 1])
            nc.sync.dma_start(out=out_d[ts(t, P), :], in_=o_sb)


def _hilo(x):
    """Split fp32 array into bf16 (hi, lo) with x ~= hi + lo."""
    import ml_dtypes

    hi = x.astype(ml_dtypes.bfloat16)
    lo = (x - hi.astype(np.float32)).astype(ml_dtypes.bfloat16)
    return np.ascontiguousarray(hi), np.ascontiguousarray(lo)


def make_in_maps(Q, K, V, Wq, Wk, Wv, Wo):
    """Host-side sharding: transpose activations, slice weights per head."""
    scale = 1.0 / np.sqrt(Wq.shape[-1])
    QTH, QTL = _hilo(np.ascontiguousarray(Q.T.astype(np.float32)))
    KTH, KTL = _hilo(np.ascontiguousarray(K.T.astype(np.float32)))
    VT = np.ascontiguousarray(V.T.astype(np.float32))
    d = Wq.shape[-1]
    in_maps = []
    for h in range(Wq.shape[0]):
        wqh, wql = _hilo(Wq[h].astype(np.float32) * scale)
        wkh, wkl = _hilo(Wk[h].astype(np.float32))
        in_maps.append({
            "QTH": QTH, "QTL": QTL, "KTH": KTH, "KTL": KTL, "VT": VT,
            "wqh": wqh, "wql": wql, "wkh": wkh, "wkl": wkl,
            "wv": np.ascontiguousarray(Wv[h].astype(np.float32)),
            "wo": np.ascontiguousarray(Wo[h * d:(h + 1) * d, :].astype(np.float32)),
        })
    return in_maps


_CACHE = {}


def _build_and_compile(n=N, dim=DIM, d=D, num_cores=H, repeats=1):
    import concourse.bass as bass
    import concourse.mybir as mybir
    import concourse.tile as tile
    from concourse import bacc

    key = (n, dim, d, num_cores, repeats)
    if key in _CACHE:
        return _CACHE[key]
    nc = bacc.Bacc("TRN2", target_bir_lowering=False, debug=False,
                   num_devices=num_cores)
    f32 = mybir.dt.float32
    bf16 = mybir.dt.bfloat16
    ins = {}
    for name in ("QTH", "QTL", "KTH", "KTL"):
        ins[name] = nc.dram_tensor(name, [dim, n], bf16, kind="ExternalInput").ap()
    ins["VT"] = nc.dram_tensor("VT", [dim, n], f32, kind="ExternalInput").ap()
    for name in ("wqh", "wql", "wkh", "wkl"):
        ins[name] = nc.dram_tensor(name, [dim, d], bf16, kind="ExternalInput").ap()
    ins["wv"] = nc.dram_tensor("wv", [dim, d], f32, kind="ExternalInput").ap()
    ins["wo"] = nc.dram_tensor("wo", [d, dim], f32, kind="ExternalInput").ap()
    outs = {"out": nc.dram_tensor("out", [n, dim], f32, kind="ExternalOutput").ap()}
    with tile.TileContext(nc) as tc:
        for _rep in range(repeats):
            with ExitStack() as ctx:
                build_head_kernel(ctx, tc, outs, ins, n=n, dim=dim, d=d)
    nc.compile()
    _CACHE[key] = nc
    return nc


def run_on_hw(in_maps, trace=False, **kwargs):
    from concourse.bass_utils import run_bass_kernel_spmd

    nc = _build_and_compile(num_cores=len(in_maps))
    return run_bass_kernel_spmd(nc, in_maps, core_ids=list(range(len(in_maps))),
                                trace=trace, **kwargs)


def kernel(Q, K, V, Wq, Wk, Wv, Wo):
    in_maps = make_in_maps(np.asarray(Q), np.asarray(K), np.asarray(V),
                           np.asarray(Wq), np.asarray(Wk), np.asarray(Wv),
                           np.asarray(Wo))
    res = run_on_hw(in_maps)
    out = np.zeros((N, DIM), dtype=np.float64)
    for r in res.results:
        out += r["out"].astype(np.float64)
    return out.astype(np.float32)


if __name__ == "__main__":
    rng = np.random.default_rng(0)
    inputs = {
        "Q": rng.standard_normal((N, DIM), dtype=np.float32),
        "K": rng.standard_normal((N, DIM), dtype=np.float32),
        "V": rng.standard_normal((N, DIM), dtype=np.float32),
        "Wq": rng.random((H, DIM, D), dtype=np.float32),
        "Wk": rng.random((H, DIM, D), dtype=np.float32),
        "Wv": rng.random((H, DIM, D), dtype=np.float32),
        "Wo": rng.random((DIM, DIM), dtype=np.float32),
    }
    out = kernel(**inputs)
    print(out.shape, out.dtype, np.abs(out).max())


# revision 28
# speedup vs baseline: 3.2029x; 1.0882x over previous
"""Multi-head attention on 8 Trainium2 NeuronCores (head-parallel).

Problem: Q,K,V [4096,512] fp32; Wq/Wk/Wv [8,512,64]; Wo [512,512].
  out = concat_h(softmax(QWq_h (KWk_h)^T / sqrt(64)) VWv_h) @ Wo

Sharding: one head per core. Each core computes its head end-to-end plus
its slice of the output projection (out_h @ Wo[64h:64h+64, :]); the host
sums the 8 partial [4096,512] outputs.

Per-core pipeline (n = 4096 queries, m = 4096 keys, d = 64):
  P1  projections (fp32 matmul). q and k are split hi/lo into bf16 pairs
      (q = q_hi + q_lo exactly in fp32; each part bf16) so the score
      matmuls can run at bf16 rate with ~fp32 accuracy via
      s = k_hi q_hi + k_lo q_hi + k_hi q_lo (the dropped lo*lo term is
      ~1e-3 absolute on logits of O(700)). v is evicted to bf16 tiles
      [128, d+1] with a constant ones column: the ones column makes the
      attn.V matmul also produce the softmax denominator.
      1/sqrt(d) is folded into Wq on the host.
  P2  per 512-query chunk, software-pipelined one chunk ahead:
      stats pass (bf16, 2-way row-packed): natural-layout scores
        [n-tile, m] -> per-row max (DVE reduce over PSUM); row maxes are
        DMA-scattered into row 64 of the q_hi operand.
      main pass (bf16 hi/lo): transposed scores; the hi*hi matmul carries
        a 65th contraction row (k side = -1, q side = rowmax) so the PSUM
        result is qk^T - rowmax directly; the two cross terms are K=64
        and run 2-way row-packed across m-tiles. ACT exp evicts
        PSUM -> SBUF bf16 attn^T.
      attn.V (bf16): accumulate outT [d+1, 512] in PSUM over all 32
        m-tiles; row d is the softmax sum.
  P3  Wo (bf16): partial[n-tile,512] = outT^T @ wo, scaled by 1/sum per
      query row (DVE per-partition scalar) on PSUM->SBUF eviction.

The double scores computation exists because softmax needs the query
index on partitions (per-partition reduce) while the attn.V matmul needs
the key index on partitions; scores are computed in both layouts (the
stats one only feeds the max, so it can be sloppy) rather than
transposing a 64MB attn matrix on-chip.

Row maxes ride in bf16: softmax is shift-invariant, so subtracting a
max that is off by <3 only scales exp values by <e^3, which the
self-consistent denominator (computed from the same bf16 attn weights)
cancels exactly.
"""

from contextlib import ExitStack

import numpy as np

N = 4096
DIM = 512
H = 8
D = 64
P = 128
CH = 512  # query columns per era (chunk)


def build_head_kernel(ctx, tc, outs, ins, n=N, dim=DIM, d=D):
    import concourse.bass as bass
    import concourse.mybir as mybir
    from concourse.bass import ts, ds

    nc = tc.nc
    f32 = mybir.dt.float32
    bf16 = mybir.dt.bfloat16
    AF = mybir.ActivationFunctionType

    KC = dim // P      # projection contraction chunks (4)
    NT = n // P        # 128-row tiles of n (= m tiles) (32)
    NCH = n // CH      # eras (8)
    NTC = CH // P      # n-tiles per era (4)
    MC = n // 512      # 512-wide m-chunks for the stats pass (8)
    PAIRS = MC // 2    # packed stats pairs per n-tile (4)
    GRP = NT // 2      # main groups per era, 2 m-tiles each (16)
    assert n % 1024 == 0 and dim % P == 0 and CH == 512

    qth_d, qtl_d = ins["QTH"], ins["QTL"]
    kth_d, ktl_d = ins["KTH"], ins["KTL"]
    vt_d = ins["VT"]
    wqh_d, wql_d = ins["wqh"], ins["wql"]
    wkh_d, wkl_d = ins["wkh"], ins["wkl"]
    wv_d, wo_d = ins["wv"], ins["wo"]
    out_d = outs["out"]

    singles = ctx.enter_context(tc.tile_pool(name="singles", bufs=1))

    # Persistent SBUF tensors.  *dup tiles hold the same data relocated to
    # partitions 64..127 so pairs of K<=64 matmuls can run concurrently in
    # distinct PE row-groups (tile_position row packing).
    Ah_ev = singles.tile([d + 1, n], bf16)  # q_hi; row d = rowmax (even eras)
    Ah_od = singles.tile([d + 1, n], bf16)  # q_hi; row d = rowmax (odd eras)
    Al = singles.tile([d, n], bf16)         # q_lo
    Bh = singles.tile([d + 1, n], bf16)     # k_hi; row d = -1
    Bl = singles.tile([d, n], bf16)         # k_lo
    qdup = singles.tile([P, n], bf16)       # rows 64..127 = q_hi
    kdup = singles.tile([P, n], bf16)       # rows 64..127 = k_hi
    qldup = singles.tile([P, n], bf16)      # rows 64..127 = q_lo
    kldup = singles.tile([P, n], bf16)      # rows 64..127 = k_lo
    v_sb = singles.tile([P, NT, d + 1], bf16)  # v tiles + ones column
    outT = singles.tile([d, n], bf16)       # attn_u @ v
    sumx = singles.tile([1, n], f32)        # softmax denominators
    rsum = singles.tile([P, NT], f32)       # sumexp gathered per n-tile
    rinv = singles.tile([P, NT], f32)
    wqh_sb = singles.tile([P, KC, d], bf16)
    wql_sb = singles.tile([P, KC, d], bf16)
    wkh_sb = singles.tile([P, KC, d], bf16)
    wkl_sb = singles.tile([P, KC, d], bf16)
    wv_sb = singles.tile([P, KC, d], f32)
    wo_sb = singles.tile([d, dim], f32)
    wo_bf = singles.tile([d, dim], bf16)

    # ---- constants / weight loads ----
    for w_sb, w_d in ((wqh_sb, wqh_d), (wql_sb, wql_d),
                      (wkh_sb, wkh_d), (wkl_sb, wkl_d), (wv_sb, wv_d)):
        nc.sync.dma_start(out=w_sb, in_=w_d.rearrange("(c p) e -> p c e", p=P))
    nc.sync.dma_start(out=wo_sb, in_=wo_d)
    nc.vector.tensor_copy(wo_bf, wo_sb)
    nc.vector.memset(Bh[d:d + 1, :], -1.0)
    nc.vector.memset(v_sb[:, :, d:d + 1], 1.0)

    # stats machinery: PSUM pool opens before P1 so chunk-0 stats can run
    # inside the (DMA-bound) projection phase as its k-chunks land
    st_pool = ctx.enter_context(tc.tile_pool(name="st_ps_pool", bufs=1, space="PSUM"))
    nmax_pool = ctx.enter_context(tc.tile_pool(name="nmax_pool", bufs=5))

    # stats nmax tiles are per-n-tile scratch; chunk-0 emission is p-major
    # (pair index advances as k-projection chunks complete), so all NTC nmax
    # tiles are live at once -- nmax_pool bufs covers NTC + the cmax tile
    nmax_tiles = {}

    def stats_item(c, g):
        """One 2-way row-packed pair of natural-layout score matmuls."""
        j, p = divmod(g, PAIRS)
        gt = c * NTC + j  # global n-tile
        if p == 0:
            nmax_tiles[j] = nmax_pool.tile([P, PAIRS], bf16, tag="nmax",
                                           name="nmax")
        st_ps = st_pool.tile([P, 1024], f32)
        nc.tensor.matmul(st_ps[:, 0:512], lhsT=Ah_ev[0:d, ts(gt, P)],
                         rhs=Bh[0:d, ts(2 * p, 512)], start=True, stop=True)
        nc.tensor.matmul(st_ps[:, 512:1024], lhsT=qdup[d:2 * d, ts(gt, P)],
                         rhs=kdup[d:2 * d, ts(2 * p + 1, 512)], start=True, stop=True)
        nc.vector.reduce_max(nmax_tiles[j][:, p:p + 1], st_ps,
                             axis=mybir.AxisListType.X)
        if p == PAIRS - 1:
            if j == 0:  # first finished n-tile of this chunk: alloc gather buf
                stats_item.cmax = nmax_pool.tile([P, NTC], bf16, tag="cmax")
            nc.vector.reduce_max(stats_item.cmax[:, j:j + 1], nmax_tiles[j],
                                 axis=mybir.AxisListType.X)
        if g == NTC * PAIRS - 1:
            At = Ah_ev if c % 2 == 0 else Ah_od
            # scatter per-row maxes into row d: column n = c*CH + jj*P + row
            for jj in range(NTC):
                nc.sync.dma_start(out=At[d:d + 1, ds(c * CH + jj * P, P)],
                                  in_=stats_item.cmax[:, jj:jj + 1])

    # ---- P1: projections (bf16 hi/lo), chunk-0 stats folded in ----
    NB = n // 512
    pending = []  # chunk-0 stats thunks, emitted at spaced slots for overlap

    def flush_one():
        if pending:
            pending.pop(0)()

    with tc.tile_pool(name="pstream", bufs=3) as pstream, \
         tc.tile_pool(name="pq_ps", bufs=2, space="PSUM") as pq_pool, \
         tc.tile_pool(name="pk_ps", bufs=2, space="PSUM") as pk_pool, \
         tc.tile_pool(name="pv_ps", bufs=2, space="PSUM") as pv_pool:

        def load_stream(t_d, tag, dtype, cols, nbs):
            t = pstream.tile([P, KC, cols], dtype, tag=tag, name=tag)
            nc.sync.dma_start(out=t, in_=t_d[:, nbs].rearrange("(c p) x -> p c x", p=P))
            return [t[:, kc, :] for kc in range(KC)]

        def v_tile(mt):
            vt_t = load_stream(vt_d, "vt", f32, P, ts(mt, P))
            ps_v = pv_pool.tile([P, d], f32)
            for kc in range(KC):
                nc.tensor.matmul(ps_v, lhsT=vt_t[kc], rhs=wv_sb[:, kc, :],
                                 start=(kc == 0), stop=(kc == KC - 1))
            nc.vector.tensor_copy(v_sb[:, mt, 0:d], ps_v)

        for nb in range(NB):
            nbs = ds(nb * 512, 512)
            qth_t = load_stream(qth_d, "qth", bf16, 512, nbs)
            qtl_t = load_stream(qtl_d, "qtl", bf16, 512, nbs)
            kth_t = load_stream(kth_d, "kth", bf16, 512, nbs)
            ktl_t = load_stream(ktl_d, "ktl", bf16, 512, nbs)
            flush_one()
            ps_q = pq_pool.tile([d, 512], f32)
            ps_k = pk_pool.tile([d, 512], f32)
            # q = Wq^T Q via bf16 hi/lo (lo*lo dropped)
            terms_q = [(wqh_sb, qth_t), (wqh_sb, qtl_t), (wql_sb, qth_t)]
            for i, (w, x) in enumerate(terms_q):
                for kc in range(KC):
                    nc.tensor.matmul(ps_q, lhsT=w[:, kc, :], rhs=x[kc],
                                     start=(i == 0 and kc == 0),
                                     stop=(i == 2 and kc == KC - 1))
            nc.scalar.copy(Ah_ev[0:d, nbs], ps_q)                  # hi = bf16(q)
            nc.vector.tensor_sub(Al[:, nbs], ps_q, Ah_ev[0:d, nbs])  # lo = q - hi
            flush_one()
            terms_k = [(wkh_sb, kth_t), (wkh_sb, ktl_t), (wkl_sb, kth_t)]
            for i, (w, x) in enumerate(terms_k):
                for kc in range(KC):
                    nc.tensor.matmul(ps_k, lhsT=w[:, kc, :], rhs=x[kc],
                                     start=(i == 0 and kc == 0),
                                     stop=(i == 2 and kc == KC - 1))
            nc.scalar.copy(Bh[0:d, nbs], ps_k)
            nc.vector.tensor_sub(Bl[:, nbs], ps_k, Bh[0:d, nbs])
            flush_one()
            # relocate this chunk's hi/lo copies to partitions 64..127
            # (SBUF->SBUF DMA can cross partitions; compute engines cannot)
            nc.sync.dma_start(out=qdup[d:2 * d, nbs], in_=Ah_ev[0:d, nbs])
            nc.sync.dma_start(out=kdup[d:2 * d, nbs], in_=Bh[0:d, nbs])
            nc.sync.dma_start(out=qldup[d:2 * d, nbs], in_=Al[:, nbs])
            nc.sync.dma_start(out=kldup[d:2 * d, nbs], in_=Bl[:, nbs])
            v_tile(2 * nb)
            flush_one()
            v_tile(2 * nb + 1)
            flush_one()
            if nb % 2 == 1:
                # k-chunks 2p, 2p+1 (p = nb//2) are now projected+relocated:
                # queue the chunk-0 stats pairs that contract against them
                p = nb // 2
                for j in range(NTC):
                    pending.append(lambda j=j, p=p: stats_item(0, j * PAIRS + p))
        for mt in range(2 * NB, NT):
            v_tile(mt)
            flush_one()
        while pending:
            flush_one()
        # odd-era copy of q_hi (separate tile so era c+1's rowmax scatter
        # never WARs era c's score matmul reads)
        nc.sync.dma_start(out=Ah_od[0:d, :], in_=Ah_ev[0:d, :])

    # ---- P2: stats (chunk c+1) interleaved with main (chunk c) ----
    with tc.tile_pool(name="sc_ps_pool", bufs=2, space="PSUM") as sc_pool, \
         tc.tile_pool(name="av_ps_pool", bufs=2, space="PSUM") as av_pool, \
         tc.tile_pool(name="att_pool", bufs=5) as att_pool:

        def wo_tile(t, ps_tile, o_sb):
            """Output-projection for n-tile t, scaled by 1/sumexp on eviction."""
            nc.tensor.matmul(ps_tile, lhsT=outT[:, ts(t, P)], rhs=wo_bf,
                             start=True, stop=True)
            # alternate eviction engine so neither DVE nor ACT serializes
            if t % 2 == 0:
                nc.vector.tensor_scalar_mul(o_sb, ps_tile, rinv[:, t:t + 1])
            else:
                nc.scalar.mul(o_sb, ps_tile, rinv[:, t:t + 1])
            nc.sync.dma_start(out=out_d[ts(t, P), :], in_=o_sb)

        def era(c):
            """Main pass for chunk c; stats for chunk c+1 interleaved."""
            At = Ah_ev if c % 2 == 0 else Ah_od
            cs = ds(c * CH, CH)
            r_hi65 = At[:, cs]        # [d+1, 512], row d = rowmax
            r_hi = At[0:d, cs]
            r_lo = Al[:, cs]
            r_hi_b = qdup[d:2 * d, cs]
            r_lo_b = qldup[d:2 * d, cs]
            av_ps = av_pool.tile([d + 1, 512], f32, tag="av")
            att_fifo = []  # (att_tile, g) awaiting attn.V, deferred 2 groups

            def emit_av(att_t, g):
                nc.tensor.matmul(av_ps, lhsT=v_sb[:, 2 * g, :], rhs=att_t[:, 0:512],
                                 start=(g == 0), stop=False)
                nc.tensor.matmul(av_ps, lhsT=v_sb[:, 2 * g + 1, :], rhs=att_t[:, 512:1024],
                                 start=False, stop=(g == GRP - 1))

            for g in range(GRP):
                if c + 1 < NCH and g < NTC * PAIRS:
                    stats_item(c + 1, g)
                mta, mtb = ts(2 * g, P), ts(2 * g + 1, P)
                sc_ps = sc_pool.tile([P, 1024], f32, tag="sc")
                att_t = att_pool.tile([P, 1024], bf16, tag="att")
                sa, sb = sc_ps[:, 0:512], sc_ps[:, 512:1024]
                # hi*hi with the rowmax-subtraction row (K=65, unpackable)
                nc.tensor.matmul(sa, lhsT=Bh[:, mta], rhs=r_hi65, start=True, stop=False)
                nc.tensor.matmul(sb, lhsT=Bh[:, mtb], rhs=r_hi65, start=True, stop=False)
                # cross terms, 2-way row-packed (rows 0..63 / 64..127)
                nc.tensor.matmul(sa, lhsT=Bl[:, mta], rhs=r_hi, start=False, stop=False)
                nc.tensor.matmul(sb, lhsT=kldup[d:2 * d, mtb], rhs=r_hi_b,
                                 start=False, stop=False)
                nc.tensor.matmul(sa, lhsT=Bh[0:d, mta], rhs=r_lo, start=False, stop=True)
                nc.tensor.matmul(sb, lhsT=kdup[d:2 * d, mtb], rhs=r_lo_b,
                                 start=False, stop=True)
                nc.scalar.activation(att_t, sc_ps, AF.Exp)
                # defer attn.V two groups so the exp it reads is long done
                # even when row-packing makes PE outpace ACT on real hardware
                att_fifo.append((att_t, g))
                if len(att_fifo) > 2:
                    emit_av(*att_fifo.pop(0))
                if c == NCH - 1 and 2 * g < NT - NTC:
                    # the stats PSUM banks are idle in the last era (there is
                    # no chunk-NCH stats pass): run the earlier chunks' output
                    # projection there, overlapped with this era's compute
                    wops = st_pool.tile([P, 1024], f32, tag="st_ps", name="wops")
                    for i in range(2):
                        o_sb = att_pool.tile([P, dim], f32, tag="o_early",
                                             name="o_early")
                        wo_tile(2 * g + i, wops[:, i * 512:(i + 1) * 512], o_sb)
            for item in att_fifo:
                emit_av(*item)
            # evict attn_u @ v (bf16) and the sumexp row (fp32), then gather
            # the per-n-tile denominators
            nc.scalar.copy(outT[:, cs], av_ps[0:d, :])
            nc.scalar.copy(sumx[:, cs], av_ps[d:d + 1, :])
            for jj in range(NTC):
                nc.sync.dma_start(out=rsum[:, c * NTC + jj:c * NTC + jj + 1],
                                  in_=sumx[:, ds(c * CH + jj * P, P)])
            nc.vector.reciprocal(rinv[:, ds(c * NTC, NTC)], rsum[:, ds(c * NTC, NTC)])

        for c in range(NCH):
            era(c)

        # ---- P3 tail: last chunk's output projection (the rest ran in the
        # final era on the idle stats banks) ----
        for t in range(NT - NTC, NT):
            wops = st_pool.tile([P, 1024], f32, tag="st_ps", name="wops")
            o_sb = att_pool.tile([P, dim], f32, tag="o_early", name="o_early")
            wo_tile(t, wops[:, 0:512], o_sb)


def _hilo(x):
    """Split fp32 array into bf16 (hi, lo) with x ~= hi + lo."""
    import ml_dtypes

    hi = x.astype(ml_dtypes.bfloat16)
    lo = (x - hi.astype(np.float32)).astype(ml_dtypes.bfloat16)
    return np.ascontiguousarray(hi), np.ascontiguousarray(lo)


def make_in_maps(Q, K, V, Wq, Wk, Wv, Wo):
    """Host-side sharding: transpose activations, slice weights per head."""
    scale = 1.0 / np.sqrt(Wq.shape[-1])
    QTH, QTL = _hilo(np.ascontiguousarray(Q.T.astype(np.float32)))
    KTH, KTL = _hilo(np.ascontiguousarray(K.T.astype(np.float32)))
    VT = np.ascontiguousarray(V.T.astype(np.float32))
    d = Wq.shape[-1]
    in_maps = []
    for h in range(Wq.shape[0]):
        wqh, wql = _hilo(Wq[h].astype(np.float32) * scale)
        wkh, wkl = _hilo(Wk[h].astype(np.float32))
        in_maps.append({
            "QTH": QTH, "QTL": QTL, "KTH": KTH, "KTL": KTL, "VT": VT,
            "wqh": wqh, "wql": wql, "wkh": wkh, "wkl": wkl,
            "wv": np.ascontiguousarray(Wv[h].astype(np.float32)),
            "wo": np.ascontiguousarray(Wo[h * d:(h + 1) * d, :].astype(np.float32)),
        })
    return in_maps


_CACHE = {}


def _build_and_compile(n=N, dim=DIM, d=D, num_cores=H, repeats=1):
    import concourse.bass as bass
    import concourse.mybir as mybir
    import concourse.tile as tile
    from concourse import bacc

    key = (n, dim, d, num_cores, repeats)
    if key in _CACHE:
        return _CACHE[key]
    nc = bacc.Bacc("TRN2", target_bir_lowering=False, debug=False,
                   num_devices=num_cores)
    f32 = mybir.dt.float32
    bf16 = mybir.dt.bfloat16
    ins = {}
    for name in ("QTH", "QTL", "KTH", "KTL"):
        ins[name] = nc.dram_tensor(name, [dim, n], bf16, kind="ExternalInput").ap()
    ins["VT"] = nc.dram_tensor("VT", [dim, n], f32, kind="ExternalInput").ap()
    for name in ("wqh", "wql", "wkh", "wkl"):
        ins[name] = nc.dram_tensor(name, [dim, d], bf16, kind="ExternalInput").ap()
    ins["wv"] = nc.dram_tensor("wv", [dim, d], f32, kind="ExternalInput").ap()
    ins["wo"] = nc.dram_tensor("wo", [d, dim], f32, kind="ExternalInput").ap()
    outs = {"out": nc.dram_tensor("out", [n, dim], f32, kind="ExternalOutput").ap()}
    with tile.TileContext(nc) as tc:
        for _rep in range(repeats):
            with ExitStack() as ctx:
                build_head_kernel(ctx, tc, outs, ins, n=n, dim=dim, d=d)
    nc.compile()
    _CACHE[key] = nc
    return nc


def run_on_hw(in_maps, trace=False, **kwargs):
    from concourse.bass_utils import run_bass_kernel_spmd

    nc = _build_and_compile(num_cores=len(in_maps))
    return run_bass_kernel_spmd(nc, in_maps, core_ids=list(range(len(in_maps))),
                                trace=trace, **kwargs)


def kernel(Q, K, V, Wq, Wk, Wv, Wo):
    in_maps = make_in_maps(np.asarray(Q), np.asarray(K), np.asarray(V),
                           np.asarray(Wq), np.asarray(Wk), np.asarray(Wv),
                           np.asarray(Wo))
    res = run_on_hw(in_maps)
    out = np.zeros((N, DIM), dtype=np.float64)
    for r in res.results:
        out += r["out"].astype(np.float64)
    return out.astype(np.float32)


if __name__ == "__main__":
    rng = np.random.default_rng(0)
    inputs = {
        "Q": rng.standard_normal((N, DIM), dtype=np.float32),
        "K": rng.standard_normal((N, DIM), dtype=np.float32),
        "V": rng.standard_normal((N, DIM), dtype=np.float32),
        "Wq": rng.random((H, DIM, D), dtype=np.float32),
        "Wk": rng.random((H, DIM, D), dtype=np.float32),
        "Wv": rng.random((H, DIM, D), dtype=np.float32),
        "Wo": rng.random((DIM, DIM), dtype=np.float32),
    }
    out = kernel(**inputs)
    print(out.shape, out.dtype, np.abs(out).max())
